# revision 10
# baseline (speedup 1.0000x reference)
"""NeighborSample Trainium2 kernel, v19: all-16-slot 3-queue balanced schedule.

Input  x:   (8, 64, 64, 192) f32
Output:     (8*64*64, 5, 5, 192) f32 — out[b*4096 + h*64 + w, i, j, c] =
            x[b, h+i-2, w+j-2, c] (zero-padded).

Pure DMA, data-parallel over batch (1 sample per NeuronCore). Input is
zero-padded on the host to (68, 68, 192).

HW model (measured via ntff profiles of v16/v17):
- The 16 SDMA engines are a shared ~425 GB/s ceiling (~26.5 GB/s
  each, slice-rate 25.7 GB/s for 3840 B store descriptors). Queue
  count beyond 2 adds nothing; per-queue rate is a demand-weighted
  share (SWDGE's 2-desc packets get a 2:1 round-robin share).
- SWDGE (gpsimd, Q0) is desc-gen limited to ~55 desc/us ≈ 212 GB/s,
  and a dma_start's doorbell only rings after ALL its descriptors are
  generated (first transfer ~17 us for a 2048-desc DMA).
- DRAM->DRAM counts once against the engine ceiling; HBM sustained
  640 GB/s total in v16, not binding.
- Stores with <16 engine slots (c8/c24/the old c30+c2 tail) strand
  engine capacity — v17 measured a ~20-30% aggregate loss in phases
  dominated by such DMAs. Everything here is c32 (16 slots x 2 rows).

Total SDMA bytes 82.2 MB -> ~194 us floor + ~15 us ramp/preamble.
v16 = 243.9 us lost ~25 us to a c=2 2-engine tail and ~20 us to Q0
idling after t=100 us (it only had 15.7 MB).

v18 layout (10 c32 stores, 2 halves x 5 shifts):
- gpsimd/SWDGE: i=0 and i=4 for both halves, DRAM->DRAM from padded x
  (zero deps, 31.4 MB ≈ its gen-limited fair share over the run).
- sync (h0) / scalar (h1): load rows 1-32 (c32) + rows 33-34 (c2,
  dedicated sems), then stores i=1 (gate la1), i=2, i=3 (gate la3).
  25.4 MB each. No cross-engine dependencies anywhere.

DMA fan-out rule (measured): outermost count c splits over n = (largest
divisor of c <= 16) engine slots, c/n consecutive rows per slot. Sem
increments total +16 per DMA; a dedicated sem per gating load (a wait
can otherwise be satisfied by another DMA's increments).
"""

import sys

for _p in ("/opt/trn_rl_repo",):
    if _p not in sys.path:
        sys.path.insert(0, _p)

import numpy as np

import concourse.bass as bass
import concourse.mybir as mybir
from concourse.bass_utils import run_bass_kernel_spmd

B = 8
H = W = 64
C = 192
K = 5
PAD = 2
HP = H + 2 * PAD     # 68 padded rows
WP = W + 2 * PAD     # 68 padded cols
ROW = WP * C         # 13056 elems per partition (one padded row)
WIN = K * C          # 960: one (h, w, i) output chunk
OUT_W = K * K * C    # 4800
OUT_H = W * OUT_W    # 307200
HH = H // 2          # 32 output rows per half


def _store(eng, out, buf, half, i, h0, cnt):
    """SBUF->DRAM store: shift i, this half's local output rows [h0, h0+cnt)."""
    return eng.dma_start(
        out=bass.AP(
            out,
            (HH * half + h0) * OUT_H + i * WIN,
            [[OUT_H, cnt], [OUT_W, W], [1, WIN]],
        ),
        in_=bass.AP(
            buf, (64 * half + i + h0) * ROW, [[ROW, cnt], [C, W], [1, WIN]]
        ),
    )


def _load(eng, x, buf, half, r0, cnt):
    """Load this half's padded rows [r0, r0+cnt) into partitions."""
    return eng.dma_start(
        out=bass.AP(buf, (64 * half + r0) * ROW, [[ROW, cnt], [1, ROW]]),
        in_=bass.AP(x, (HH * half + r0) * ROW, [[ROW, cnt], [1, ROW]]),
    )


def _load_colsplit(eng, x, buf, half, r0, nrows, nchunk):
    """Load nrows rows as nchunk column-chunks per row-pair so the DMA
    fans out over nchunk engine slots instead of nrows (v18's c2 load
    rode engines 64/65 only, dragging their store backlog ~7 us past
    the others). The partition dim keeps its pitch stride (the BIR
    verifier rejects strides that cross partitions off-pitch); the
    chunk dim is outermost so fan-out follows it."""
    chunk = ROW // nchunk
    return eng.dma_start(
        out=bass.AP(
            buf,
            (64 * half + r0) * ROW,
            [[chunk, nchunk], [ROW, nrows], [1, chunk]],
        ),
        in_=bass.AP(
            x,
            (HH * half + r0) * ROW,
            [[chunk, nchunk], [ROW, nrows], [1, chunk]],
        ),
    )


def _dram_store(eng, x, out, half, i):
    """Full-half shift i direct from padded x in DRAM — no SBUF, no deps."""
    return eng.dma_start(
        out=bass.AP(
            out,
            (HH * half) * OUT_H + i * WIN,
            [[OUT_H, HH], [OUT_W, W], [1, WIN]],
        ),
        in_=bass.AP(
            x,
            (HH * half + i) * ROW,
            [[ROW, HH], [C, W], [1, WIN]],
        ),
    )


def _emit_ring(eng, x, out, buf, la1, la2, la3, dsem, half):
    _load(eng, x, buf, half, 1, 16).then_inc(la1, 16)
    _load(eng, x, buf, half, 17, 16).then_inc(la2, 16)
    _load(eng, x, buf, half, 33, 2).then_inc(la3, 16)
    eng.wait_ge(la1, 16)
    _store(eng, out, buf, half, 1, 0, 16).then_inc(dsem, 16)
    eng.wait_ge(la2, 16)
    _store(eng, out, buf, half, 1, 16, 16).then_inc(dsem, 16)
    eng.wait_ge(la3, 16)
    _store(eng, out, buf, half, 2, 0, HH).then_inc(dsem, 16)
    _store(eng, out, buf, half, 3, 0, HH).then_inc(dsem, 16)
    eng.wait_ge(dsem, 16 * 4)


def build_nc() -> bass.Bass:
    nc = bass.Bass()
    x = nc.declare_dram_parameter("x", [HP, WP, C], mybir.dt.float32, isOutput=False)
    out = nc.declare_dram_parameter(
        "out", [H, W, K, K, C], mybir.dt.float32, isOutput=True
    )

    with (
        nc.Block() as block,
        nc.semaphore("la1") as la1,
        nc.semaphore("la2") as la2,
        nc.semaphore("la3") as la3,
        nc.semaphore("d_a") as d_a,
        nc.semaphore("lb1") as lb1,
        nc.semaphore("lb2") as lb2,
        nc.semaphore("lb3") as lb3,
        nc.semaphore("d_b") as d_b,
        nc.semaphore("d_g") as d_g,
        nc.sbuf_tensor("buf", [128, ROW], mybir.dt.float32) as buf,
    ):

        @block.sync
        def _(sync):
            _emit_ring(sync, x, out, buf, la1, la2, la3, d_a, 0)

        @block.scalar
        def _(scalar):
            _emit_ring(scalar, x, out, buf, lb1, lb2, lb3, d_b, 1)

        @block.gpsimd
        def _(gpsimd):
            _dram_store(gpsimd, x, out, 0, 0).then_inc(d_g, 16)
            _dram_store(gpsimd, x, out, 1, 0).then_inc(d_g, 16)
            _dram_store(gpsimd, x, out, 0, 4).then_inc(d_g, 16)
            _dram_store(gpsimd, x, out, 1, 4).then_inc(d_g, 16)
            gpsimd.wait_ge(d_g, 16 * 4)

    return nc


_NC_CACHE = None


def prep_in_maps(x):
    xp = np.zeros((B, HP, WP, C), dtype=np.float32)
    xp[:, PAD : PAD + H, PAD : PAD + W, :] = x
    return [{"x": np.ascontiguousarray(xp[i])} for i in range(B)]


def kernel(x) -> np.ndarray:
    global _NC_CACHE
    x = np.asarray(x, dtype=np.float32)
    assert x.shape == (B, H, W, C), x.shape
    if _NC_CACHE is None:
        _NC_CACHE = build_nc()
    in_maps = prep_in_maps(x)
    res = run_bass_kernel_spmd(_NC_CACHE, in_maps, list(range(B)))
    outs = [res.results[i]["out"].reshape(H * W, K, K, C) for i in range(B)]
    return np.concatenate(outs, axis=0)


# revision 17
# speedup vs baseline: 1.1424x; 1.1424x over previous
"""NeighborSample Trainium2 kernel, v22: engine-balanced 3-queue schedule.

Input  x:   (8, 64, 64, 192) f32
Output:     (8*64*64, 5, 5, 192) f32 — out[b*4096 + h*64 + w, i, j, c] =
            x[b, h+i-2, w+j-2, c] (zero-padded).

Pure DMA, data-parallel over batch (1 sample per NeuronCore). Input is
zero-padded on the host to (68, 68, 192).

HW model (measured across v16-v21 ntff profiles):
- 16 SDMA engines (E64..E79), shared ~425 GB/s aggregate (~26.5 GB/s
  each). >=2 busy queues saturate it; SWDGE's 2-desc packets take a
  2:1:1 round-robin share (212 GB/s) vs the two HWDGE rings.
- Every dma_start MUST carry sync info ("DGE must have sync info"),
  and its per-engine sem-update descriptor is a WAW fence: the engine
  stalls until its outstanding writes ack. Fence count == DMA count,
  so keep DMAs few and large (v18: 14 DMAs -> 219 us; v19: 19 -> 256;
  v17: 25 -> 313). balance_dma_aps rejects >3-dim APs, so shifts
  cannot be merged into one DMA.
- Loads (DRAM->SBUF) are read-latency-bound at ~7-8 GB/s/engine/queue;
  stores and DRAM->DRAM run ~26 GB/s/engine. SWDGE's first transfer
  lands ~15 us in (desc-gen), regardless of DMA size.
- Engines E78/E79 are ~10-15% slower; with uniform per-engine bytes
  they straggle ~3-5 us past the rest. A c28 store (fan-out = 14
  slots x 2 rows) sheds exactly E78/E79, so giving each ring one c28
  store rebalances per-engine *time*. DRAM->DRAM APs have no
  partition-dim rules, so Q0 absorbs the shed rows fanned over w.
- HBM tops out ~716 GB/s; only ~212 GB/s of D2D (read+write) fits
  alongside the stores, so exactly one queue (Q0) runs D2D.

Layout (82.2 MB SDMA total -> ~194 us floor):
- gpsimd/SWDGE Q0 (33.4 MB): D2D from padded x: i=0 h0+h1 (c32),
  i=2 rows 28-31 h0+h1 (w-fanned, 16 slots), i=4 h0+h1 (c32).
- sync (h0) / scalar (h1) rings (24.4 MB each): load rows 1-32 (c32,
  sem la1) + rows 33-34 (c2, sem la3); then [la1] i=1 c32 and i=2
  rows 0-27 c28 (needs only rows 2-29), then [la3] i=3 c32.
DMA fan-out rule (measured): outermost count c splits over n = largest
divisor of c <= 16 slots, c/n consecutive rows per slot.
"""

import sys

for _p in ("/opt/trn_rl_repo",):
    if _p not in sys.path:
        sys.path.insert(0, _p)

import numpy as np

import concourse.bass as bass
import concourse.mybir as mybir
from concourse.bass_utils import run_bass_kernel_spmd

B = 8
H = W = 64
C = 192
K = 5
PAD = 2
HP = H + 2 * PAD     # 68 padded rows
WP = W + 2 * PAD     # 68 padded cols
ROW = WP * C         # 13056 elems per partition (one padded row)
WIN = K * C          # 960: one (h, w, i) output chunk
OUT_W = K * K * C    # 4800
OUT_H = W * OUT_W    # 307200
HH = H // 2          # 32 output rows per half


def _store(eng, out, buf, half, i, h0, cnt):
    """SBUF->DRAM store: shift i, this half's local output rows [h0, h0+cnt)."""
    return eng.dma_start(
        out=bass.AP(
            out,
            (HH * half + h0) * OUT_H + i * WIN,
            [[OUT_H, cnt], [OUT_W, W], [1, WIN]],
        ),
        in_=bass.AP(
            buf, (64 * half + i + h0) * ROW, [[ROW, cnt], [C, W], [1, WIN]]
        ),
    )


def _load(eng, x, buf, half, r0, cnt):
    """Load this half's padded rows [r0, r0+cnt) into partitions."""
    return eng.dma_start(
        out=bass.AP(buf, (64 * half + r0) * ROW, [[ROW, cnt], [1, ROW]]),
        in_=bass.AP(x, (HH * half + r0) * ROW, [[ROW, cnt], [1, ROW]]),
    )


def _dram_store(eng, x, out, half, i):
    """Full-half shift i direct from padded x in DRAM — no SBUF, no deps."""
    return eng.dma_start(
        out=bass.AP(
            out,
            (HH * half) * OUT_H + i * WIN,
            [[OUT_H, HH], [OUT_W, W], [1, WIN]],
        ),
        in_=bass.AP(
            x,
            (HH * half + i) * ROW,
            [[ROW, HH], [C, W], [1, WIN]],
        ),
    )


def _dram_store_rows_wfan(eng, x, out, half, i, h0, cnt):
    """Shift i, rows [h0, h0+cnt), direct from x, fanned over w so all
    16 engine slots share it (D2D APs have no partition rules)."""
    return eng.dma_start(
        out=bass.AP(
            out,
            (HH * half + h0) * OUT_H + i * WIN,
            [[OUT_W, W], [OUT_H, cnt], [1, WIN]],
        ),
        in_=bass.AP(
            x,
            (HH * half + i + h0) * ROW,
            [[C, W], [ROW, cnt], [1, WIN]],
        ),
    )


def _emit_ring(eng, x, out, buf, la1, la3, dsem, half):
    _load(eng, x, buf, half, 1, 32).then_inc(la1, 16)
    _load(eng, x, buf, half, 33, 2).then_inc(la3, 16)
    eng.wait_ge(la1, 16)
    _store(eng, out, buf, half, 1, 0, HH).then_inc(dsem, 16)
    _store(eng, out, buf, half, 2, 0, 28).then_inc(dsem, 16)
    eng.wait_ge(la3, 16)
    _store(eng, out, buf, half, 3, 0, HH).then_inc(dsem, 16)
    eng.wait_ge(dsem, 16 * 3)


def build_nc() -> bass.Bass:
    nc = bass.Bass()
    x = nc.declare_dram_parameter("x", [HP, WP, C], mybir.dt.float32, isOutput=False)
    out = nc.declare_dram_parameter(
        "out", [H, W, K, K, C], mybir.dt.float32, isOutput=True
    )

    with (
        nc.Block() as block,
        nc.semaphore("la1") as la1,
        nc.semaphore("la3") as la3,
        nc.semaphore("d_a") as d_a,
        nc.semaphore("lb1") as lb1,
        nc.semaphore("lb3") as lb3,
        nc.semaphore("d_b") as d_b,
        nc.semaphore("d_g") as d_g,
        nc.sbuf_tensor("buf", [128, ROW], mybir.dt.float32) as buf,
    ):

        @block.sync
        def _(sync):
            _emit_ring(sync, x, out, buf, la1, la3, d_a, 0)

        @block.scalar
        def _(scalar):
            _emit_ring(scalar, x, out, buf, lb1, lb3, d_b, 1)

        @block.gpsimd
        def _(gpsimd):
            _dram_store(gpsimd, x, out, 0, 0).then_inc(d_g, 16)
            _dram_store(gpsimd, x, out, 1, 0).then_inc(d_g, 16)
            _dram_store_rows_wfan(gpsimd, x, out, 0, 2, 28, 4).then_inc(d_g, 16)
            _dram_store_rows_wfan(gpsimd, x, out, 1, 2, 28, 4).then_inc(d_g, 16)
            _dram_store(gpsimd, x, out, 0, 4).then_inc(d_g, 16)
            _dram_store(gpsimd, x, out, 1, 4).then_inc(d_g, 16)
            gpsimd.wait_ge(d_g, 16 * 6)

    return nc


_NC_CACHE = None


def prep_in_maps(x):
    xp = np.zeros((B, HP, WP, C), dtype=np.float32)
    xp[:, PAD : PAD + H, PAD : PAD + W, :] = x
    return [{"x": np.ascontiguousarray(xp[i])} for i in range(B)]


def kernel(x) -> np.ndarray:
    global _NC_CACHE
    x = np.asarray(x, dtype=np.float32)
    assert x.shape == (B, H, W, C), x.shape
    if _NC_CACHE is None:
        _NC_CACHE = build_nc()
    in_maps = prep_in_maps(x)
    res = run_bass_kernel_spmd(_NC_CACHE, in_maps, list(range(B)))
    outs = [res.results[i]["out"].reshape(H * W, K, K, C) for i in range(B)]
    return np.concatenate(outs, axis=0)
